# revision 25
# baseline (speedup 1.0000x reference)
"""Causal multi-head flash-attention block (QKV proj + attention + out proj)
for Trainium2, distributed over 8 NeuronCores.

Sharding: data-parallel over batch (B=4) x tensor-parallel over head groups
(16 heads -> 2 groups of 8). Core c handles batch c//2, head group c%2.
Each core computes a partial output projection (its 8 heads' contribution);
the host sums the two partials per batch and adds the bias.

v3 design (trace-driven):
  - Scores psum tiles are per k-block [128, 1024] (h0 cols 0:512, h1
    512:1024) with bufs=2: the two heads' score matmuls are row-tiled
    (K=64 at partitions 0:64 / 64:128) into different psum banks and
    co-execute on the PE array; exp runs per k-block on ACT.
  - Causal masking is folded into the scores psum via one accumulating
    K=128 matmul per (diagonal block, head): lhsT = -BIG*I, rhs = a wide
    shifted lower-triangle indicator. exp then yields exact zeros; no
    DVE mask multiplies. (A single full-array matmul: row-tiled halves
    would co-execute into the same psum bank, which the HW forbids.)
  - Softmax normalize: psum->sbuf copy, denominator row partition-shift
    via tiny sbuf->sbuf DMA, reciprocal_approx_fast (DVE), GpSimd
    partition_broadcast, DVE multiply.
  - All non-attention PE work (QKV projection, V strips, output
    projection) is cut into ~2.5us mini-chunks (one psum slot each) in a
    deadline-tagged filler queue, drained one chunk per score-group so
    the PE never idles and psum slot pressure stays smooth.
  - psum->sbuf evacuation copies stay on DVE explicitly (routing them
    to ACT via nc.any measurably slowed both exp and the PE clock).
"""

import numpy as np
import ml_dtypes

import concourse.bass as bass
import concourse.bacc as bacc
import concourse.mybir as mybir
import concourse.tile as tile
from concourse.bass_utils import run_bass_kernel_spmd

F32 = mybir.dt.float32
BF16 = mybir.dt.bfloat16
EXP = mybir.ActivationFunctionType.Exp

# Problem constants (hardcoded per contract)
B, S, C = 4, 2048, 1024
NH, D = 16, 64
SCALE = D ** -0.5
N_CORES = 8
HG = NH // 2          # heads per core (head group)
NPAIR = HG // 2       # head pairs per core
CCH = C // 128        # contraction chunks for QKV proj
SC = S // 128         # s-chunks (also k-blocks count)
NQT = S // 512        # q-tiles of 512
GW = C // 2           # group width of qkv output (8 heads * 64)
NEG = -29952.0        # causal mask additive constant (exactly bf16)
END = (NPAIR, NQT, 0)  # deadline meaning "flush at end"


def build_nc(diag_restrict=True):
    nc = bacc.Bacc("TRN2", target_bir_lowering=False, debug=False)

    xT = nc.dram_tensor("xT", [C, S], BF16, kind="ExternalInput")
    wq = nc.dram_tensor("wq", [C, GW], BF16, kind="ExternalInput")
    wk = nc.dram_tensor("wk", [C, GW], BF16, kind="ExternalInput")
    wv = nc.dram_tensor("wv", [C, GW], BF16, kind="ExternalInput")
    wp = nc.dram_tensor("wp", [GW, C], BF16, kind="ExternalInput")
    negI = nc.dram_tensor("negI", [128, 128], BF16, kind="ExternalInput")
    tri = nc.dram_tensor("tri", [128, 896], BF16, kind="ExternalInput")
    out = nc.dram_tensor("out", [S, C], F32, kind="ExternalOutput")

    with tile.TileContext(nc) as tc:
        with (
            tc.tile_pool(name="const", bufs=1) as cpool,
            tc.tile_pool(name="qk", bufs=2) as qkpool,
            tc.tile_pool(name="pt", bufs=8) as ptpool,
            tc.tile_pool(name="work", bufs=2) as wpool,
            tc.tile_pool(name="sps", bufs=2, space="PSUM") as spspool,
            tc.tile_pool(name="otp", bufs=2, space="PSUM") as otpool,
            tc.tile_pool(name="mm", bufs=2, space="PSUM") as mmpool,
        ):
            # ---- constant/persistent tiles + input DMAs ----
            xt_sb, wq_sb, wk_sb, wv_sb = [], [], [], []
            for cc in range(CCH):
                t = cpool.tile([128, S], BF16, tag=f"xt{cc}", name=f"xt{cc}")
                for q4 in range(4):
                    nc.sync.dma_start(t[:, 512 * q4:512 * (q4 + 1)],
                                      xT[128 * cc:128 * (cc + 1),
                                         512 * q4:512 * (q4 + 1)])
                xt_sb.append(t)
                t = cpool.tile([128, GW], BF16, tag=f"wv{cc}", name=f"wv{cc}")
                nc.sync.dma_start(t[:], wv[128 * cc:128 * (cc + 1), :])
                wv_sb.append(t)
            for cc in range(CCH):
                for name, dram, lst in (("wq", wq, wq_sb), ("wk", wk, wk_sb)):
                    t = cpool.tile([128, GW], BF16, tag=f"{name}{cc}",
                                   name=f"{name}{cc}")
                    nc.sync.dma_start(t[:], dram[128 * cc:128 * (cc + 1), :])
                    lst.append(t)
            wp_sb = []
            for p in range(NPAIR):
                t = cpool.tile([128, C], BF16, tag=f"wp{p}", name=f"wp{p}")
                nc.sync.dma_start(t[:], wp[128 * p:128 * (p + 1), :])
                wp_sb.append(t)
            negI_sb = cpool.tile([128, 128], BF16, tag="negI", name="negI_t")
            nc.sync.dma_start(negI_sb[:], negI[:, :])
            tri_sb = cpool.tile([128, 896], BF16, tag="tri", name="tri_t")
            nc.sync.dma_start(tri_sb[:], tri[:, :])
            # preload the ACT exp table set while input DMAs run
            actwarm = cpool.tile([1, 8], F32, tag="actwarm", name="actwarm")
            nc.vector.memset(actwarm[:], 0.0)
            nc.scalar.activation(actwarm[:], actwarm[:], EXP)

            # O^T normalized, per head pair: head0 partitions 0:64,
            # head1 partitions 64:128 (layout = rows of w_proj)
            otn_sb = [cpool.tile([128, S], BF16, tag=f"otn{p}", name=f"otn{p}")
                      for p in range(NPAIR)]

            # ---- V = x @ wv in natural [s, d] layout, + ones column ----
            vt_sb = [cpool.tile([128, 65 * HG], BF16, tag=f"vt{sc}",
                                name=f"vt{sc}")
                     for sc in range(SC)]

            def emit_v_strip(sc):
                vt = vt_sb[sc]
                nc.gpsimd.memset(vt[:], 1.0)
                ps = mmpool.tile([128, GW], F32, tag="mm", name="vps")
                for cc in range(CCH):
                    nc.tensor.matmul(
                        ps[:], xt_sb[cc][:, 128 * sc:128 * (sc + 1)],
                        wv_sb[cc][:], start=(cc == 0), stop=(cc == CCH - 1))
                vt_v = vt[:, :].rearrange("p (h d) -> p h d", h=HG)[:, :, 0:64]
                ps_v = ps[:, :].rearrange("p (h d) -> p h d", h=HG)
                nc.vector.tensor_copy(vt_v, ps_v)

            # ---- QK projection mini-chunk: one (pair, q|k, s-tile) ----
            qt_t = [None] * NPAIR
            kt_t = [None] * NPAIR

            def emit_qk_chunk(p, which, st):
                if which == "q":
                    if qt_t[p] is None:
                        qt_t[p] = qkpool.tile([128, S], BF16, tag="qt",
                                              name=f"qt{p}")
                    w_sb, dst = wq_sb, qt_t[p]
                else:
                    if kt_t[p] is None:
                        kt_t[p] = qkpool.tile([128, S], BF16, tag="kt",
                                              name=f"kt{p}")
                    w_sb, dst = wk_sb, kt_t[p]
                ps = mmpool.tile([128, 512], F32, tag="mm", name="qkps")
                for cc in range(CCH):
                    nc.tensor.matmul(
                        ps[:],
                        w_sb[cc][:, 128 * p:128 * (p + 1)],
                        xt_sb[cc][:, 512 * st:512 * (st + 1)],
                        start=(cc == 0), stop=(cc == CCH - 1))
                nc.vector.tensor_copy(dst[:, 512 * st:512 * (st + 1)], ps[:])

            # ---- output projection mini-chunk: one (s-chunk, half) ----
            outst_t = {}

            def emit_outproj(sc, half):
                pp = mmpool.tile([128, 512], F32, tag="mm", name="pp")
                for p in range(NPAIR):
                    nc.tensor.matmul(
                        pp[:],
                        otn_sb[p][:, 128 * sc:128 * (sc + 1)],
                        wp_sb[p][:, 512 * half:512 * (half + 1)],
                        start=(p == 0), stop=(p == NPAIR - 1))
                if sc not in outst_t:
                    outst_t[sc] = wpool.tile([128, C], F32, tag="outst",
                                             name=f"outst{sc}")
                o = outst_t[sc]
                nc.vector.tensor_copy(o[:, 512 * half:512 * (half + 1)], pp[:])
                if half == 1:
                    nc.sync.dma_start(out[128 * sc:128 * (sc + 1), :], o[:])
                    del outst_t[sc]

            # ---- filler queue: (deadline, closure), deadline = (p, j, g)
            fillq = []

            def drain_fillers(now, budget):
                i = 0
                while i < len(fillq):
                    dl, fn = fillq[i]
                    if dl <= now:
                        fn()
                        fillq.pop(i)
                        budget -= 1
                    else:
                        i += 1
                while budget > 0 and fillq:
                    dl, fn = fillq.pop(0)
                    fn()
                    budget -= 1

            # ---- attention inner machinery ----
            def emit_scores_kb(p, j, kb):
                qt, kt = qt_t[p], kt_t[p]
                sp = spspool.tile([128, 1024], F32, tag="sp", name="sp")
                diag = kb >= 4 * j
                ot_ = 128 * (kb - 4 * j) if diag else 0  # true block offset
                o = ot_ if diag_restrict else 0          # written col range
                for h in range(2):
                    hsl = slice(64 * h, 64 * (h + 1))
                    nc.tensor.matmul(
                        sp[:, 512 * h + o:512 * (h + 1)],
                        kt[hsl, 128 * kb:128 * (kb + 1)],
                        qt[hsl, 512 * j + o:512 * (j + 1)],
                        start=True, stop=not diag)
                if diag:
                    sh = 384 - (ot_ - o)
                    w = 512 - o
                    for h in range(2):
                        csl = slice(512 * h + o, 512 * (h + 1))
                        # single full-array K=128 matmul: row-tiled halves
                        # would co-execute into the same psum bank, which
                        # the HW forbids
                        nc.tensor.matmul(
                            sp[:, csl], negI_sb[:, :],
                            tri_sb[:, sh:sh + w],
                            start=False, stop=True)
                pt = ptpool.tile([128, 1024], BF16, tag="pt", name="pt")
                nc.scalar.activation(pt[:], sp[:], EXP, scale=SCALE)
                return pt

            def emit_av_group(p, j, g, pts, ot, nkb):
                for kb in (2 * g, 2 * g + 1):
                    o = 128 * (kb - 4 * j) if kb >= 4 * j else 0
                    for h in range(2):
                        nc.tensor.matmul(
                            ot[h][:, o:512],
                            vt_sb[kb][:, 65 * (2 * p + h):
                                      65 * (2 * p + h) + 65],
                            pts[kb][:, 512 * h + o:512 * (h + 1)],
                            start=(kb == 0), stop=(kb == nkb - 1))

            def emit_normalize(p, j, ot):
                qsl = slice(512 * j, 512 * (j + 1))
                s64 = wpool.tile([65, 1024], F32, tag="s64", name="s64")
                for h in range(2):
                    nc.vector.tensor_copy(s64[:, 512 * h:512 * (h + 1)],
                                          ot[h][:, :])
                # partition-shift the two denominator rows to partition 0
                dsh = wpool.tile([1, 1024], F32, tag="dsh", name="dsh")
                nc.sync.dma_start(dsh[0:1, :], s64[64:65, :])
                inv = wpool.tile([1, 1024], F32, tag="inv", name="inv")
                nc.vector.reciprocal_approx_fast(inv[0:1, :], dsh[0:1, :])
                for h in range(2):
                    bcs = wpool.tile([64, 512], F32, tag="bcs", name="bcs")
                    nc.gpsimd.partition_broadcast(
                        bcs[:], inv[0:1, 512 * h:512 * (h + 1)])
                    if h == 0:
                        nc.vector.tensor_mul(otn_sb[p][0:64, qsl],
                                             s64[0:64, 0:512], bcs[:])
                    else:
                        oth = wpool.tile([64, 512], BF16, tag="oth",
                                         name="oth")
                        nc.vector.tensor_mul(oth[:], s64[0:64, 512:1024],
                                             bcs[:])
                        # partition-shifting copy into rows 64:128
                        nc.sync.dma_start(otn_sb[p][64:128, qsl], oth[:])

            # ---- main schedule ----
            # upfront: V strips 0..3, pair-0 qt/kt for j=0
            for sc in range(4):
                emit_v_strip(sc)
            emit_qk_chunk(0, "q", 0)
            emit_qk_chunk(0, "k", 0)
            # queue the rest of pair 0's QK with deadlines
            for st in range(1, NQT):
                fillq.append(((0, st, 0),
                              lambda st=st: emit_qk_chunk(0, "q", st)))
                fillq.append(((0, st, 0),
                              lambda st=st: emit_qk_chunk(0, "k", st)))

            for p in range(NPAIR):
                # queue next pair's QK chunks (due before (p+1, st, 0))
                if p + 1 < NPAIR:
                    for st in range(NQT):
                        fillq.append(((p + 1, st, 0),
                                      lambda p=p, st=st:
                                      emit_qk_chunk(p + 1, "q", st)))
                        fillq.append(((p + 1, st, 0),
                                      lambda p=p, st=st:
                                      emit_qk_chunk(p + 1, "k", st)))
                if p == 0:
                    # V strips 4..15: strip s first consumed by AV in
                    # (0, j=s//4) at group ~s//2+3 (lag-2 pipeline)
                    for s in range(4, SC):
                        jj = s // 4
                        gg = min(s // 2 + 3, 2 * (jj + 1) - 1)
                        fillq.append(((0, jj, gg),
                                      lambda s=s: emit_v_strip(s)))
                    fillq.sort(key=lambda e: e[0])

                for j in range(NQT):
                    nkb = 4 * (j + 1)
                    ot = [otpool.tile([65, 512], F32, tag="ot", name="ot")
                          for _ in range(2)]
                    pts = {}
                    pending = []
                    for g in range(nkb // 2):
                        drain_fillers((p, j, g), 1)
                        for kb in (2 * g, 2 * g + 1):
                            pts[kb] = emit_scores_kb(p, j, kb)
                        pending.append(g)
                        if len(pending) > 2:
                            emit_av_group(p, j, pending.pop(0), pts, ot, nkb)
                    for g in pending:
                        emit_av_group(p, j, g, pts, ot, nkb)
                    emit_normalize(p, j, ot)
                    drain_fillers((p, j, 99), 1)
                    if p == 3:
                        # out-projection for this j's s-chunks becomes legal
                        # once all pairs have normalized j
                        for sc in range(4 * j, 4 * (j + 1)):
                            for half in range(2):
                                fillq.append(
                                    (END, lambda sc=sc, half=half:
                                     emit_outproj(sc, half)))
            drain_fillers(END, len(fillq) + 1)

    nc.compile()
    return nc


_NC_CACHE = None


def _get_nc():
    global _NC_CACHE
    if _NC_CACHE is None:
        _NC_CACHE = build_nc()
    return _NC_CACHE


def make_in_maps(x, w_qkv, w_proj):
    """Shard full inputs into the 8 per-core input dicts."""
    bf = ml_dtypes.bfloat16
    negI = (NEG * np.eye(128, dtype=np.float32)).astype(bf)
    tri = ((np.arange(896)[None, :] - 384) < np.arange(128)[:, None]).astype(bf)
    in_maps = []
    for core in range(N_CORES):
        b, g = core // 2, core % 2
        gsl = slice(GW * g, GW * (g + 1))
        in_maps.append({
            "xT": np.ascontiguousarray(x[b].T).astype(bf),
            "wq": np.ascontiguousarray(w_qkv[:, 0 * C:1 * C][:, gsl]).astype(bf),
            "wk": np.ascontiguousarray(w_qkv[:, 1 * C:2 * C][:, gsl]).astype(bf),
            "wv": np.ascontiguousarray(w_qkv[:, 2 * C:3 * C][:, gsl]).astype(bf),
            "wp": np.ascontiguousarray(w_proj[gsl, :]).astype(bf),
            "negI": negI,
            "tri": tri,
        })
    return in_maps


def kernel(x, w_qkv, w_proj, b_proj, _profile=False):
    import os
    if not _profile:
        # the NTFF trace path needs modules absent from this image;
        # make sure an inherited BASS_TRACE can't route us into it
        os.environ["BASS_NEVER_TRACE"] = "1"
    else:
        os.environ.pop("BASS_NEVER_TRACE", None)
    x = np.asarray(x, np.float32)
    w_qkv = np.asarray(w_qkv, np.float32)
    w_proj = np.asarray(w_proj, np.float32)
    b_proj = np.asarray(b_proj, np.float32)

    nc = _get_nc()
    in_maps = make_in_maps(x, w_qkv, w_proj)
    res = run_bass_kernel_spmd(nc, in_maps, core_ids=list(range(N_CORES)),
                               trace=_profile)
    partials = [res.results[c]["out"] for c in range(N_CORES)]
    out = np.empty((B, S, C), np.float32)
    for b in range(B):
        out[b] = partials[2 * b] + partials[2 * b + 1] + b_proj
    if _profile:
        return out, res
    return out


# revision 26
# speedup vs baseline: 1.0539x; 1.0539x over previous
"""Causal multi-head flash-attention block (QKV proj + attention + out proj)
for Trainium2, distributed over 8 NeuronCores.

Sharding: data-parallel over batch (B=4) x tensor-parallel over head groups
(16 heads -> 2 groups of 8). Core c handles batch c//2, head group c%2.
Each core computes a partial output projection (its 8 heads' contribution);
the host sums the two partials per batch and adds the bias.

v3 design (trace-driven):
  - Scores psum tiles are per k-block [128, 1024] (h0 cols 0:512, h1
    512:1024) with bufs=2: the two heads' score matmuls are row-tiled
    (K=64 at partitions 0:64 / 64:128) into different psum banks and
    co-execute on the PE array; exp runs per k-block on ACT.
  - Causal masking is folded into the scores psum via one accumulating
    K=128 matmul per (diagonal block, head): lhsT = -BIG*I, rhs = a wide
    shifted lower-triangle indicator. exp then yields exact zeros; no
    DVE mask multiplies. (A single full-array matmul: row-tiled halves
    would co-execute into the same psum bank, which the HW forbids.)
  - Softmax normalize: psum->sbuf copy, denominator row partition-shift
    via tiny sbuf->sbuf DMA, reciprocal_approx_fast (DVE), GpSimd
    partition_broadcast, DVE multiply.
  - All non-attention PE work (QKV projection, V strips, output
    projection) is cut into ~2.5us mini-chunks (one psum slot each) in a
    deadline-tagged filler queue, drained one chunk per score-group so
    the PE never idles and psum slot pressure stays smooth.
  - psum->sbuf evacuation copies stay on DVE explicitly (routing them
    to ACT via nc.any measurably slowed both exp and the PE clock).
"""

import numpy as np
import ml_dtypes

import concourse.bass as bass
import concourse.bacc as bacc
import concourse.mybir as mybir
import concourse.tile as tile
from concourse.bass_utils import run_bass_kernel_spmd

F32 = mybir.dt.float32
BF16 = mybir.dt.bfloat16
EXP = mybir.ActivationFunctionType.Exp

# Problem constants (hardcoded per contract)
B, S, C = 4, 2048, 1024
NH, D = 16, 64
SCALE = D ** -0.5
N_CORES = 8
HG = NH // 2          # heads per core (head group)
NPAIR = HG // 2       # head pairs per core
CCH = C // 128        # contraction chunks for QKV proj
SC = S // 128         # s-chunks (also k-blocks count)
NQT = S // 512        # q-tiles of 512
GW = C // 2           # group width of qkv output (8 heads * 64)
NEG = -29952.0        # causal mask additive constant (exactly bf16)
END = (NPAIR, NQT, 0)  # deadline meaning "flush at end"


def build_nc(diag_restrict=True):
    nc = bacc.Bacc("TRN2", target_bir_lowering=False, debug=False)

    xT = nc.dram_tensor("xT", [C, S], BF16, kind="ExternalInput")
    wq = nc.dram_tensor("wq", [C, GW], BF16, kind="ExternalInput")
    wk = nc.dram_tensor("wk", [C, GW], BF16, kind="ExternalInput")
    wv = nc.dram_tensor("wv", [C, GW], BF16, kind="ExternalInput")
    wp = nc.dram_tensor("wp", [GW, C], BF16, kind="ExternalInput")
    negI = nc.dram_tensor("negI", [128, 128], BF16, kind="ExternalInput")
    tri = nc.dram_tensor("tri", [128, 896], BF16, kind="ExternalInput")
    out = nc.dram_tensor("out", [S, C], F32, kind="ExternalOutput")

    with tile.TileContext(nc) as tc:
        with (
            tc.tile_pool(name="const", bufs=1) as cpool,
            tc.tile_pool(name="qk", bufs=2) as qkpool,
            tc.tile_pool(name="pt", bufs=8) as ptpool,
            tc.tile_pool(name="work", bufs=2) as wpool,
            tc.tile_pool(name="sps", bufs=2, space="PSUM") as spspool,
            tc.tile_pool(name="otp", bufs=2, space="PSUM") as otpool,
            tc.tile_pool(name="mm", bufs=2, space="PSUM") as mmpool,
        ):
            # ---- constant/persistent tiles + input DMAs ----
            xt_sb, wq_sb, wk_sb, wv_sb = [], [], [], []
            for cc in range(CCH):
                t = cpool.tile([128, S], BF16, tag=f"xt{cc}", name=f"xt{cc}")
                nc.sync.dma_start(t[:], xT[128 * cc:128 * (cc + 1), :])
                xt_sb.append(t)
                t = cpool.tile([128, GW], BF16, tag=f"wv{cc}", name=f"wv{cc}")
                nc.sync.dma_start(t[:], wv[128 * cc:128 * (cc + 1), :])
                wv_sb.append(t)
            for cc in range(CCH):
                for name, dram, lst in (("wq", wq, wq_sb), ("wk", wk, wk_sb)):
                    t = cpool.tile([128, GW], BF16, tag=f"{name}{cc}",
                                   name=f"{name}{cc}")
                    nc.sync.dma_start(t[:], dram[128 * cc:128 * (cc + 1), :])
                    lst.append(t)
            wp_sb = []
            for p in range(NPAIR):
                t = cpool.tile([128, C], BF16, tag=f"wp{p}", name=f"wp{p}")
                nc.sync.dma_start(t[:], wp[128 * p:128 * (p + 1), :])
                wp_sb.append(t)
            negI_sb = cpool.tile([128, 128], BF16, tag="negI", name="negI_t")
            nc.sync.dma_start(negI_sb[:], negI[:, :])
            tri_sb = cpool.tile([128, 896], BF16, tag="tri", name="tri_t")
            nc.sync.dma_start(tri_sb[:], tri[:, :])
            # preload the ACT exp table set while input DMAs run
            actwarm = cpool.tile([1, 8], F32, tag="actwarm", name="actwarm")
            nc.vector.memset(actwarm[:], 0.0)
            nc.scalar.activation(actwarm[:], actwarm[:], EXP)

            # O^T normalized, per head pair: head0 partitions 0:64,
            # head1 partitions 64:128 (layout = rows of w_proj)
            otn_sb = [cpool.tile([128, S], BF16, tag=f"otn{p}", name=f"otn{p}")
                      for p in range(NPAIR)]

            # ---- V = x @ wv in natural [s, d] layout, + ones column ----
            vt_sb = [cpool.tile([128, 65 * HG], BF16, tag=f"vt{sc}",
                                name=f"vt{sc}")
                     for sc in range(SC)]

            def emit_v_strip(sc):
                vt = vt_sb[sc]
                nc.gpsimd.memset(vt[:], 1.0)
                ps = mmpool.tile([128, GW], F32, tag="mm", name="vps")
                for cc in range(CCH):
                    nc.tensor.matmul(
                        ps[:], xt_sb[cc][:, 128 * sc:128 * (sc + 1)],
                        wv_sb[cc][:], start=(cc == 0), stop=(cc == CCH - 1))
                vt_v = vt[:, :].rearrange("p (h d) -> p h d", h=HG)[:, :, 0:64]
                ps_v = ps[:, :].rearrange("p (h d) -> p h d", h=HG)
                nc.vector.tensor_copy(vt_v, ps_v)

            # ---- QK projection mini-chunk: one (pair, q|k, s-tile) ----
            qt_t = [None] * NPAIR
            kt_t = [None] * NPAIR

            def emit_qk_chunk(p, which, st):
                if which == "q":
                    if qt_t[p] is None:
                        qt_t[p] = qkpool.tile([128, S], BF16, tag="qt",
                                              name=f"qt{p}")
                    w_sb, dst = wq_sb, qt_t[p]
                else:
                    if kt_t[p] is None:
                        kt_t[p] = qkpool.tile([128, S], BF16, tag="kt",
                                              name=f"kt{p}")
                    w_sb, dst = wk_sb, kt_t[p]
                ps = mmpool.tile([128, 512], F32, tag="mm", name="qkps")
                for cc in range(CCH):
                    nc.tensor.matmul(
                        ps[:],
                        w_sb[cc][:, 128 * p:128 * (p + 1)],
                        xt_sb[cc][:, 512 * st:512 * (st + 1)],
                        start=(cc == 0), stop=(cc == CCH - 1))
                nc.vector.tensor_copy(dst[:, 512 * st:512 * (st + 1)], ps[:])

            # ---- output projection mini-chunk: one (s-chunk, half) ----
            outst_t = {}

            def emit_outproj(sc, half):
                pp = mmpool.tile([128, 512], F32, tag="mm", name="pp")
                for p in range(NPAIR):
                    nc.tensor.matmul(
                        pp[:],
                        otn_sb[p][:, 128 * sc:128 * (sc + 1)],
                        wp_sb[p][:, 512 * half:512 * (half + 1)],
                        start=(p == 0), stop=(p == NPAIR - 1))
                if sc not in outst_t:
                    outst_t[sc] = wpool.tile([128, C], F32, tag="outst",
                                             name=f"outst{sc}")
                o = outst_t[sc]
                nc.vector.tensor_copy(o[:, 512 * half:512 * (half + 1)], pp[:])
                if half == 1:
                    nc.sync.dma_start(out[128 * sc:128 * (sc + 1), :], o[:])
                    del outst_t[sc]

            # ---- filler queue: (deadline, closure), deadline = (p, j, g)
            fillq = []

            def drain_fillers(now, budget):
                i = 0
                while i < len(fillq):
                    dl, fn = fillq[i]
                    if dl <= now:
                        fn()
                        fillq.pop(i)
                        budget -= 1
                    else:
                        i += 1
                while budget > 0 and fillq:
                    dl, fn = fillq.pop(0)
                    fn()
                    budget -= 1

            # ---- attention inner machinery ----
            def emit_scores_kb(p, j, kb):
                qt, kt = qt_t[p], kt_t[p]
                sp = spspool.tile([128, 1024], F32, tag="sp", name="sp")
                diag = kb >= 4 * j
                ot_ = 128 * (kb - 4 * j) if diag else 0  # true block offset
                o = ot_ if diag_restrict else 0          # written col range
                for h in range(2):
                    hsl = slice(64 * h, 64 * (h + 1))
                    nc.tensor.matmul(
                        sp[:, 512 * h + o:512 * (h + 1)],
                        kt[hsl, 128 * kb:128 * (kb + 1)],
                        qt[hsl, 512 * j + o:512 * (j + 1)],
                        start=True, stop=not diag)
                if diag:
                    sh = 384 - (ot_ - o)
                    w = 512 - o
                    for h in range(2):
                        csl = slice(512 * h + o, 512 * (h + 1))
                        # single full-array K=128 matmul: row-tiled halves
                        # would co-execute into the same psum bank, which
                        # the HW forbids
                        nc.tensor.matmul(
                            sp[:, csl], negI_sb[:, :],
                            tri_sb[:, sh:sh + w],
                            start=False, stop=True)
                pt = ptpool.tile([128, 1024], BF16, tag="pt", name="pt")
                nc.scalar.activation(pt[:], sp[:], EXP, scale=SCALE)
                return pt

            def emit_av_group(p, j, g, pts, ot, nkb):
                for kb in (2 * g, 2 * g + 1):
                    o = 128 * (kb - 4 * j) if kb >= 4 * j else 0
                    for h in range(2):
                        nc.tensor.matmul(
                            ot[h][:, o:512],
                            vt_sb[kb][:, 65 * (2 * p + h):
                                      65 * (2 * p + h) + 65],
                            pts[kb][:, 512 * h + o:512 * (h + 1)],
                            start=(kb == 0), stop=(kb == nkb - 1))

            def emit_normalize(p, j, ot):
                qsl = slice(512 * j, 512 * (j + 1))
                s64 = wpool.tile([65, 1024], F32, tag="s64", name="s64")
                for h in range(2):
                    nc.vector.tensor_copy(s64[:, 512 * h:512 * (h + 1)],
                                          ot[h][:, :])
                # partition-shift the two denominator rows to partition 0
                dsh = wpool.tile([1, 1024], F32, tag="dsh", name="dsh")
                nc.sync.dma_start(dsh[0:1, :], s64[64:65, :])
                inv = wpool.tile([1, 1024], F32, tag="inv", name="inv")
                nc.vector.reciprocal_approx_fast(inv[0:1, :], dsh[0:1, :])
                for h in range(2):
                    bcs = wpool.tile([64, 512], F32, tag="bcs", name="bcs")
                    nc.gpsimd.partition_broadcast(
                        bcs[:], inv[0:1, 512 * h:512 * (h + 1)])
                    if h == 0:
                        nc.vector.tensor_mul(otn_sb[p][0:64, qsl],
                                             s64[0:64, 0:512], bcs[:])
                    else:
                        oth = wpool.tile([64, 512], BF16, tag="oth",
                                         name="oth")
                        nc.vector.tensor_mul(oth[:], s64[0:64, 512:1024],
                                             bcs[:])
                        # partition-shifting copy into rows 64:128
                        nc.sync.dma_start(otn_sb[p][64:128, qsl], oth[:])

            # ---- main schedule ----
            # upfront: V strips 0..3, pair-0 qt/kt for j=0
            for sc in range(4):
                emit_v_strip(sc)
            emit_qk_chunk(0, "q", 0)
            emit_qk_chunk(0, "k", 0)
            # queue the rest of pair 0's QK with deadlines
            for st in range(1, NQT):
                fillq.append(((0, st, 0),
                              lambda st=st: emit_qk_chunk(0, "q", st)))
                fillq.append(((0, st, 0),
                              lambda st=st: emit_qk_chunk(0, "k", st)))

            for p in range(NPAIR):
                # queue next pair's QK chunks (due before (p+1, st, 0))
                if p + 1 < NPAIR:
                    for st in range(NQT):
                        fillq.append(((p + 1, st, 0),
                                      lambda p=p, st=st:
                                      emit_qk_chunk(p + 1, "q", st)))
                        fillq.append(((p + 1, st, 0),
                                      lambda p=p, st=st:
                                      emit_qk_chunk(p + 1, "k", st)))
                if p == 0:
                    # V strips 4..15: strip s first consumed by AV in
                    # (0, j=s//4) at group ~s//2+3 (lag-2 pipeline)
                    for s in range(4, SC):
                        jj = s // 4
                        gg = min(s // 2 + 3, 2 * (jj + 1) - 1)
                        fillq.append(((0, jj, gg),
                                      lambda s=s: emit_v_strip(s)))
                    fillq.sort(key=lambda e: e[0])

                for j in range(NQT):
                    nkb = 4 * (j + 1)
                    ot = [otpool.tile([65, 512], F32, tag="ot", name="ot")
                          for _ in range(2)]
                    pts = {}
                    pending = []
                    for g in range(nkb // 2):
                        drain_fillers((p, j, g), 1)
                        for kb in (2 * g, 2 * g + 1):
                            pts[kb] = emit_scores_kb(p, j, kb)
                        pending.append(g)
                        if len(pending) > 2:
                            emit_av_group(p, j, pending.pop(0), pts, ot, nkb)
                    for g in pending:
                        emit_av_group(p, j, g, pts, ot, nkb)
                    emit_normalize(p, j, ot)
                    if p == 3:
                        # out-projection for this j's s-chunks becomes legal
                        # once all pairs have normalized j
                        for sc in range(4 * j, 4 * (j + 1)):
                            for half in range(2):
                                fillq.append(
                                    (END, lambda sc=sc, half=half:
                                     emit_outproj(sc, half)))
            drain_fillers(END, len(fillq) + 1)

    nc.compile()
    return nc


_NC_CACHE = None


def _get_nc():
    global _NC_CACHE
    if _NC_CACHE is None:
        _NC_CACHE = build_nc()
    return _NC_CACHE


def make_in_maps(x, w_qkv, w_proj):
    """Shard full inputs into the 8 per-core input dicts."""
    bf = ml_dtypes.bfloat16
    negI = (NEG * np.eye(128, dtype=np.float32)).astype(bf)
    tri = ((np.arange(896)[None, :] - 384) < np.arange(128)[:, None]).astype(bf)
    in_maps = []
    for core in range(N_CORES):
        b, g = core // 2, core % 2
        gsl = slice(GW * g, GW * (g + 1))
        in_maps.append({
            "xT": np.ascontiguousarray(x[b].T).astype(bf),
            "wq": np.ascontiguousarray(w_qkv[:, 0 * C:1 * C][:, gsl]).astype(bf),
            "wk": np.ascontiguousarray(w_qkv[:, 1 * C:2 * C][:, gsl]).astype(bf),
            "wv": np.ascontiguousarray(w_qkv[:, 2 * C:3 * C][:, gsl]).astype(bf),
            "wp": np.ascontiguousarray(w_proj[gsl, :]).astype(bf),
            "negI": negI,
            "tri": tri,
        })
    return in_maps


def kernel(x, w_qkv, w_proj, b_proj, _profile=False):
    import os
    if not _profile:
        # the NTFF trace path needs modules absent from this image;
        # make sure an inherited BASS_TRACE can't route us into it
        os.environ["BASS_NEVER_TRACE"] = "1"
    else:
        os.environ.pop("BASS_NEVER_TRACE", None)
    x = np.asarray(x, np.float32)
    w_qkv = np.asarray(w_qkv, np.float32)
    w_proj = np.asarray(w_proj, np.float32)
    b_proj = np.asarray(b_proj, np.float32)

    nc = _get_nc()
    in_maps = make_in_maps(x, w_qkv, w_proj)
    res = run_bass_kernel_spmd(nc, in_maps, core_ids=list(range(N_CORES)),
                               trace=_profile)
    partials = [res.results[c]["out"] for c in range(N_CORES)]
    out = np.empty((B, S, C), np.float32)
    for b in range(B):
        out[b] = partials[2 * b] + partials[2 * b + 1] + b_proj
    if _profile:
        return out, res
    return out


# revision 27
# speedup vs baseline: 1.0714x; 1.0166x over previous
"""Causal multi-head flash-attention block (QKV proj + attention + out proj)
for Trainium2, distributed over 8 NeuronCores.

Sharding: data-parallel over batch (B=4) x tensor-parallel over head groups
(16 heads -> 2 groups of 8). Core c handles batch c//2, head group c%2.
Each core computes a partial output projection (its 8 heads' contribution);
the host sums the two partials per batch and adds the bias.

v3 design (trace-driven):
  - Scores psum tiles are per k-block [128, 1024] (h0 cols 0:512, h1
    512:1024) with bufs=2: the two heads' score matmuls are row-tiled
    (K=64 at partitions 0:64 / 64:128) into different psum banks and
    co-execute on the PE array; exp runs per k-block on ACT.
  - Causal masking is folded into the scores psum via one accumulating
    K=128 matmul per (diagonal block, head): lhsT = -BIG*I, rhs = a wide
    shifted lower-triangle indicator. exp then yields exact zeros; no
    DVE mask multiplies. (A single full-array matmul: row-tiled halves
    would co-execute into the same psum bank, which the HW forbids.)
  - Softmax normalize: psum->sbuf copy, denominator row partition-shift
    via tiny sbuf->sbuf DMA, reciprocal_approx_fast (DVE), GpSimd
    partition_broadcast, DVE multiply.
  - All non-attention PE work (QKV projection, V strips, output
    projection) is cut into ~2.5us mini-chunks (one psum slot each) in a
    deadline-tagged filler queue, drained one chunk per score-group so
    the PE never idles and psum slot pressure stays smooth.
  - psum->sbuf evacuation copies stay on DVE explicitly (routing them
    to ACT via nc.any measurably slowed both exp and the PE clock).
"""

import numpy as np
import ml_dtypes

import concourse.bass as bass
import concourse.bacc as bacc
import concourse.mybir as mybir
import concourse.tile as tile
from concourse.bass_utils import run_bass_kernel_spmd

F32 = mybir.dt.float32
BF16 = mybir.dt.bfloat16
EXP = mybir.ActivationFunctionType.Exp

# Problem constants (hardcoded per contract)
B, S, C = 4, 2048, 1024
NH, D = 16, 64
SCALE = D ** -0.5
N_CORES = 8
HG = NH // 2          # heads per core (head group)
NPAIR = HG // 2       # head pairs per core
CCH = C // 128        # contraction chunks for QKV proj
SC = S // 128         # s-chunks (also k-blocks count)
NQT = S // 512        # q-tiles of 512
GW = C // 2           # group width of qkv output (8 heads * 64)
NEG = -29952.0        # causal mask additive constant (exactly bf16)
END = (NPAIR, NQT, 0)  # deadline meaning "flush at end"


def build_nc(diag_restrict=True):
    nc = bacc.Bacc("TRN2", target_bir_lowering=False, debug=False)

    xT = nc.dram_tensor("xT", [C, S], BF16, kind="ExternalInput")
    wq = nc.dram_tensor("wq", [C, GW], BF16, kind="ExternalInput")
    wk = nc.dram_tensor("wk", [C, GW], BF16, kind="ExternalInput")
    wv = nc.dram_tensor("wv", [C, GW], BF16, kind="ExternalInput")
    wp = nc.dram_tensor("wp", [GW, C], BF16, kind="ExternalInput")
    negI = nc.dram_tensor("negI", [128, 128], BF16, kind="ExternalInput")
    tri = nc.dram_tensor("tri", [128, 896], BF16, kind="ExternalInput")
    out = nc.dram_tensor("out", [S, C], F32, kind="ExternalOutput")

    with tile.TileContext(nc) as tc:
        with (
            tc.tile_pool(name="const", bufs=1) as cpool,
            tc.tile_pool(name="qk", bufs=2) as qkpool,
            tc.tile_pool(name="pt", bufs=8) as ptpool,
            tc.tile_pool(name="work", bufs=2) as wpool,
            tc.tile_pool(name="sps", bufs=2, space="PSUM") as spspool,
            tc.tile_pool(name="otp", bufs=2, space="PSUM") as otpool,
            tc.tile_pool(name="mm", bufs=2, space="PSUM") as mmpool,
        ):
            # ---- constant/persistent tiles + input DMAs ----
            xt_sb, wq_sb, wk_sb, wv_sb = [], [], [], []
            for cc in range(CCH):
                t = cpool.tile([128, S], BF16, tag=f"xt{cc}", name=f"xt{cc}")
                nc.sync.dma_start(t[:], xT[128 * cc:128 * (cc + 1), :])
                xt_sb.append(t)
                t = cpool.tile([128, GW], BF16, tag=f"wv{cc}", name=f"wv{cc}")
                nc.sync.dma_start(t[:], wv[128 * cc:128 * (cc + 1), :])
                wv_sb.append(t)
            for cc in range(CCH):
                for name, dram, lst in (("wq", wq, wq_sb), ("wk", wk, wk_sb)):
                    t = cpool.tile([128, GW], BF16, tag=f"{name}{cc}",
                                   name=f"{name}{cc}")
                    nc.sync.dma_start(t[:], dram[128 * cc:128 * (cc + 1), :])
                    lst.append(t)
            wp_sb = []
            for p in range(NPAIR):
                t = cpool.tile([128, C], BF16, tag=f"wp{p}", name=f"wp{p}")
                nc.sync.dma_start(t[:], wp[128 * p:128 * (p + 1), :])
                wp_sb.append(t)
            negI_sb = cpool.tile([128, 128], BF16, tag="negI", name="negI_t")
            nc.sync.dma_start(negI_sb[:], negI[:, :])
            tri_sb = cpool.tile([128, 896], BF16, tag="tri", name="tri_t")
            nc.sync.dma_start(tri_sb[:], tri[:, :])
            # preload the ACT exp table set while input DMAs run
            actwarm = cpool.tile([1, 8], F32, tag="actwarm", name="actwarm")
            nc.vector.memset(actwarm[:], 0.0)
            nc.scalar.activation(actwarm[:], actwarm[:], EXP)

            # O^T normalized, per head pair: head0 partitions 0:64,
            # head1 partitions 64:128 (layout = rows of w_proj)
            otn_sb = [cpool.tile([128, S], BF16, tag=f"otn{p}", name=f"otn{p}")
                      for p in range(NPAIR)]

            # ---- V = x @ wv in natural [s, d] layout, + ones column ----
            vt_sb = [cpool.tile([128, 65 * HG], BF16, tag=f"vt{sc}",
                                name=f"vt{sc}")
                     for sc in range(SC)]

            def emit_v_strip(sc):
                vt = vt_sb[sc]
                nc.gpsimd.memset(vt[:], 1.0)
                ps = mmpool.tile([128, GW], F32, tag="mm", name="vps")
                for cc in range(CCH):
                    nc.tensor.matmul(
                        ps[:], xt_sb[cc][:, 128 * sc:128 * (sc + 1)],
                        wv_sb[cc][:], start=(cc == 0), stop=(cc == CCH - 1))
                vt_v = vt[:, :].rearrange("p (h d) -> p h d", h=HG)[:, :, 0:64]
                ps_v = ps[:, :].rearrange("p (h d) -> p h d", h=HG)
                nc.vector.tensor_copy(vt_v, ps_v)

            # ---- QK projection mini-chunk: one (pair, q|k, s-tile) ----
            qt_t = [None] * NPAIR
            kt_t = [None] * NPAIR

            def emit_qk_chunk(p, which, st):
                if which == "q":
                    if qt_t[p] is None:
                        qt_t[p] = qkpool.tile([128, S], BF16, tag="qt",
                                              name=f"qt{p}")
                    w_sb, dst = wq_sb, qt_t[p]
                else:
                    if kt_t[p] is None:
                        kt_t[p] = qkpool.tile([128, S], BF16, tag="kt",
                                              name=f"kt{p}")
                    w_sb, dst = wk_sb, kt_t[p]
                ps = mmpool.tile([128, 512], F32, tag="mm", name="qkps")
                for cc in range(CCH):
                    nc.tensor.matmul(
                        ps[:],
                        w_sb[cc][:, 128 * p:128 * (p + 1)],
                        xt_sb[cc][:, 512 * st:512 * (st + 1)],
                        start=(cc == 0), stop=(cc == CCH - 1))
                nc.vector.tensor_copy(dst[:, 512 * st:512 * (st + 1)], ps[:])

            # ---- output projection mini-chunk: one (s-chunk, half) ----
            outst_t = {}

            def emit_outproj(sc, half):
                pp = mmpool.tile([128, 512], F32, tag="mm", name="pp")
                for p in range(NPAIR):
                    nc.tensor.matmul(
                        pp[:],
                        otn_sb[p][:, 128 * sc:128 * (sc + 1)],
                        wp_sb[p][:, 512 * half:512 * (half + 1)],
                        start=(p == 0), stop=(p == NPAIR - 1))
                if sc not in outst_t:
                    outst_t[sc] = wpool.tile([128, C], F32, tag="outst",
                                             name=f"outst{sc}")
                o = outst_t[sc]
                nc.vector.tensor_copy(o[:, 512 * half:512 * (half + 1)], pp[:])
                if half == 1:
                    nc.sync.dma_start(out[128 * sc:128 * (sc + 1), :], o[:])
                    del outst_t[sc]

            # ---- filler queue: (deadline, closure), deadline = (p, j, g)
            fillq = []

            def drain_fillers(now, budget):
                i = 0
                while i < len(fillq):
                    dl, fn = fillq[i]
                    if dl <= now:
                        fn()
                        fillq.pop(i)
                        budget -= 1
                    else:
                        i += 1
                while budget > 0 and fillq:
                    dl, fn = fillq.pop(0)
                    fn()
                    budget -= 1

            # ---- attention inner machinery ----
            def emit_scores_kb(p, j, kb):
                qt, kt = qt_t[p], kt_t[p]
                sp = spspool.tile([128, 1024], F32, tag="sp", name="sp")
                diag = kb >= 4 * j
                ot_ = 128 * (kb - 4 * j) if diag else 0  # true block offset
                o = ot_ if diag_restrict else 0          # written col range
                for h in range(2):
                    hsl = slice(64 * h, 64 * (h + 1))
                    nc.tensor.matmul(
                        sp[:, 512 * h + o:512 * (h + 1)],
                        kt[hsl, 128 * kb:128 * (kb + 1)],
                        qt[hsl, 512 * j + o:512 * (j + 1)],
                        start=True, stop=not diag)
                if diag:
                    # single full-array K=128 matmul: row-tiled halves
                    # would co-execute into the same psum bank, which
                    # the HW forbids. On HW the dead triangle sits entirely
                    # in the 128-col strip [ot_:ot_+128) and has_written is
                    # already set there, so an N=128 accumulate suffices
                    # (stop is sim-only; skip the bass group check). The
                    # sim build closes the full written range instead.
                    if diag_restrict:
                        for h in range(2):
                            csl = slice(512 * h + ot_, 512 * h + ot_ + 128)
                            nc.tensor.matmul(
                                sp[:, csl], negI_sb[:, :],
                                tri_sb[:, 384:512],
                                start=False, stop=True,
                                skip_group_check=True)
                    else:
                        sh = 384 - ot_
                        for h in range(2):
                            csl = slice(512 * h, 512 * (h + 1))
                            nc.tensor.matmul(
                                sp[:, csl], negI_sb[:, :],
                                tri_sb[:, sh:sh + 512],
                                start=False, stop=True)
                pt = ptpool.tile([128, 1024], BF16, tag="pt", name="pt")
                nc.scalar.activation(pt[:], sp[:], EXP, scale=SCALE)
                return pt

            def emit_av_group(p, j, g, pts, ot, nkb):
                for kb in (2 * g, 2 * g + 1):
                    o = 128 * (kb - 4 * j) if kb >= 4 * j else 0
                    for h in range(2):
                        nc.tensor.matmul(
                            ot[h][:, o:512],
                            vt_sb[kb][:, 65 * (2 * p + h):
                                      65 * (2 * p + h) + 65],
                            pts[kb][:, 512 * h + o:512 * (h + 1)],
                            start=(kb == 0), stop=(kb == nkb - 1))

            def emit_normalize(p, j, ot):
                qsl = slice(512 * j, 512 * (j + 1))
                s64 = wpool.tile([65, 1024], F32, tag="s64", name="s64")
                for h in range(2):
                    nc.vector.tensor_copy(s64[:, 512 * h:512 * (h + 1)],
                                          ot[h][:, :])
                # partition-shift the two denominator rows to partition 0
                dsh = wpool.tile([1, 1024], F32, tag="dsh", name="dsh")
                nc.sync.dma_start(dsh[0:1, :], s64[64:65, :])
                inv = wpool.tile([1, 1024], F32, tag="inv", name="inv")
                nc.vector.reciprocal_approx_fast(inv[0:1, :], dsh[0:1, :])
                for h in range(2):
                    bcs = wpool.tile([64, 512], F32, tag="bcs", name="bcs")
                    nc.gpsimd.partition_broadcast(
                        bcs[:], inv[0:1, 512 * h:512 * (h + 1)])
                    if h == 0:
                        nc.vector.tensor_mul(otn_sb[p][0:64, qsl],
                                             s64[0:64, 0:512], bcs[:])
                    else:
                        oth = wpool.tile([64, 512], BF16, tag="oth",
                                         name="oth")
                        nc.vector.tensor_mul(oth[:], s64[0:64, 512:1024],
                                             bcs[:])
                        # partition-shifting copy into rows 64:128
                        nc.sync.dma_start(otn_sb[p][64:128, qsl], oth[:])

            # ---- main schedule ----
            # upfront: V strips 0..3, pair-0 qt/kt for j=0
            for sc in range(4):
                emit_v_strip(sc)
            emit_qk_chunk(0, "q", 0)
            emit_qk_chunk(0, "k", 0)
            # queue the rest of pair 0's QK with deadlines
            for st in range(1, NQT):
                fillq.append(((0, st, 0),
                              lambda st=st: emit_qk_chunk(0, "q", st)))
                fillq.append(((0, st, 0),
                              lambda st=st: emit_qk_chunk(0, "k", st)))

            for p in range(NPAIR):
                # queue next pair's QK chunks (due before (p+1, st, 0))
                if p + 1 < NPAIR:
                    for st in range(NQT):
                        fillq.append(((p + 1, st, 0),
                                      lambda p=p, st=st:
                                      emit_qk_chunk(p + 1, "q", st)))
                        fillq.append(((p + 1, st, 0),
                                      lambda p=p, st=st:
                                      emit_qk_chunk(p + 1, "k", st)))
                if p == 0:
                    # V strips 4..15: strip s first consumed by AV in
                    # (0, j=s//4) at group ~s//2+3 (lag-2 pipeline)
                    for s in range(4, SC):
                        jj = s // 4
                        gg = min(s // 2 + 3, 2 * (jj + 1) - 1)
                        fillq.append(((0, jj, gg),
                                      lambda s=s: emit_v_strip(s)))
                    fillq.sort(key=lambda e: e[0])

                for j in range(NQT):
                    nkb = 4 * (j + 1)
                    ot = [otpool.tile([65, 512], F32, tag="ot", name="ot")
                          for _ in range(2)]
                    pts = {}
                    pending = []
                    for g in range(nkb // 2):
                        drain_fillers((p, j, g), 1)
                        for kb in (2 * g, 2 * g + 1):
                            pts[kb] = emit_scores_kb(p, j, kb)
                        pending.append(g)
                        if len(pending) > 2:
                            emit_av_group(p, j, pending.pop(0), pts, ot, nkb)
                    for g in pending:
                        emit_av_group(p, j, g, pts, ot, nkb)
                    emit_normalize(p, j, ot)
                    drain_fillers((p, j, 99), 1)
                    if p == 3:
                        # out-projection for this j's s-chunks becomes legal
                        # once all pairs have normalized j
                        for sc in range(4 * j, 4 * (j + 1)):
                            for half in range(2):
                                fillq.append(
                                    (END, lambda sc=sc, half=half:
                                     emit_outproj(sc, half)))
            drain_fillers(END, len(fillq) + 1)

    nc.compile()
    return nc


_NC_CACHE = None


def _get_nc():
    global _NC_CACHE
    if _NC_CACHE is None:
        _NC_CACHE = build_nc()
    return _NC_CACHE


def make_in_maps(x, w_qkv, w_proj):
    """Shard full inputs into the 8 per-core input dicts."""
    bf = ml_dtypes.bfloat16
    negI = (NEG * np.eye(128, dtype=np.float32)).astype(bf)
    tri = ((np.arange(896)[None, :] - 384) < np.arange(128)[:, None]).astype(bf)
    in_maps = []
    for core in range(N_CORES):
        b, g = core // 2, core % 2
        gsl = slice(GW * g, GW * (g + 1))
        in_maps.append({
            "xT": np.ascontiguousarray(x[b].T).astype(bf),
            "wq": np.ascontiguousarray(w_qkv[:, 0 * C:1 * C][:, gsl]).astype(bf),
            "wk": np.ascontiguousarray(w_qkv[:, 1 * C:2 * C][:, gsl]).astype(bf),
            "wv": np.ascontiguousarray(w_qkv[:, 2 * C:3 * C][:, gsl]).astype(bf),
            "wp": np.ascontiguousarray(w_proj[gsl, :]).astype(bf),
            "negI": negI,
            "tri": tri,
        })
    return in_maps


def kernel(x, w_qkv, w_proj, b_proj, _profile=False):
    import os
    if not _profile:
        # the NTFF trace path needs modules absent from this image;
        # make sure an inherited BASS_TRACE can't route us into it
        os.environ["BASS_NEVER_TRACE"] = "1"
    else:
        os.environ.pop("BASS_NEVER_TRACE", None)
    x = np.asarray(x, np.float32)
    w_qkv = np.asarray(w_qkv, np.float32)
    w_proj = np.asarray(w_proj, np.float32)
    b_proj = np.asarray(b_proj, np.float32)

    nc = _get_nc()
    in_maps = make_in_maps(x, w_qkv, w_proj)
    res = run_bass_kernel_spmd(nc, in_maps, core_ids=list(range(N_CORES)),
                               trace=_profile)
    partials = [res.results[c]["out"] for c in range(N_CORES)]
    out = np.empty((B, S, C), np.float32)
    for b in range(B):
        out[b] = partials[2 * b] + partials[2 * b + 1] + b_proj
    if _profile:
        return out, res
    return out


# revision 28
# speedup vs baseline: 1.0849x; 1.0127x over previous
"""Causal multi-head flash-attention block (QKV proj + attention + out proj)
for Trainium2, distributed over 8 NeuronCores.

Sharding: data-parallel over batch (B=4) x tensor-parallel over head groups
(16 heads -> 2 groups of 8). Core c handles batch c//2, head group c%2.
Each core computes a partial output projection (its 8 heads' contribution);
the host sums the two partials per batch and adds the bias.

v3 design (trace-driven):
  - Scores psum tiles are per k-block [128, 1024] (h0 cols 0:512, h1
    512:1024) with bufs=2: the two heads' score matmuls are row-tiled
    (K=64 at partitions 0:64 / 64:128) into different psum banks and
    co-execute on the PE array; exp runs per k-block on ACT.
  - Causal masking is folded into the scores psum via one accumulating
    K=128 matmul per (diagonal block, head): lhsT = -BIG*I, rhs = a wide
    shifted lower-triangle indicator. exp then yields exact zeros; no
    DVE mask multiplies. (A single full-array matmul: row-tiled halves
    would co-execute into the same psum bank, which the HW forbids.)
  - Softmax normalize: psum->sbuf copy, denominator row partition-shift
    via tiny sbuf->sbuf DMA, reciprocal_approx_fast (DVE), GpSimd
    partition_broadcast, DVE multiply.
  - All non-attention PE work (QKV projection, V strips, output
    projection) is cut into ~2.5us mini-chunks (one psum slot each) in a
    deadline-tagged filler queue, drained one chunk per score-group so
    the PE never idles and psum slot pressure stays smooth.
  - psum->sbuf evacuation copies stay on DVE explicitly (routing them
    to ACT via nc.any measurably slowed both exp and the PE clock).
"""

import numpy as np
import ml_dtypes

import concourse.bass as bass
import concourse.bacc as bacc
import concourse.mybir as mybir
import concourse.tile as tile
from concourse.bass_utils import run_bass_kernel_spmd

F32 = mybir.dt.float32
BF16 = mybir.dt.bfloat16
EXP = mybir.ActivationFunctionType.Exp

# Problem constants (hardcoded per contract)
B, S, C = 4, 2048, 1024
NH, D = 16, 64
SCALE = D ** -0.5
N_CORES = 8
HG = NH // 2          # heads per core (head group)
NPAIR = HG // 2       # head pairs per core
CCH = C // 128        # contraction chunks for QKV proj
SC = S // 128         # s-chunks (also k-blocks count)
NQT = S // 512        # q-tiles of 512
GW = C // 2           # group width of qkv output (8 heads * 64)
NEG = -29952.0        # causal mask additive constant (exactly bf16)
END = (NPAIR, NQT, 0)  # deadline meaning "flush at end"


def build_nc(diag_restrict=True):
    nc = bacc.Bacc("TRN2", target_bir_lowering=False, debug=False)

    xT = nc.dram_tensor("xT", [C, S], BF16, kind="ExternalInput")
    wq = nc.dram_tensor("wq", [C, GW], BF16, kind="ExternalInput")
    wk = nc.dram_tensor("wk", [C, GW], BF16, kind="ExternalInput")
    wv = nc.dram_tensor("wv", [C, GW], BF16, kind="ExternalInput")
    wp = nc.dram_tensor("wp", [GW, C], BF16, kind="ExternalInput")
    negI = nc.dram_tensor("negI", [128, 128], BF16, kind="ExternalInput")
    tri = nc.dram_tensor("tri", [128, 896], BF16, kind="ExternalInput")
    out = nc.dram_tensor("out", [S, C], F32, kind="ExternalOutput")

    with tile.TileContext(nc) as tc:
        with (
            tc.tile_pool(name="const", bufs=1) as cpool,
            tc.tile_pool(name="qk", bufs=2) as qkpool,
            tc.tile_pool(name="pt", bufs=8) as ptpool,
            tc.tile_pool(name="work", bufs=2) as wpool,
            tc.tile_pool(name="sps", bufs=2, space="PSUM") as spspool,
            tc.tile_pool(name="otp", bufs=2, space="PSUM") as otpool,
            tc.tile_pool(name="mm", bufs=2, space="PSUM") as mmpool,
        ):
            # ---- constant/persistent tiles + input DMAs ----
            xt_sb, wq_sb, wk_sb, wv_sb = [], [], [], []
            for cc in range(CCH):
                t = cpool.tile([128, S], BF16, tag=f"xt{cc}", name=f"xt{cc}")
                nc.sync.dma_start(t[:], xT[128 * cc:128 * (cc + 1), :])
                xt_sb.append(t)
                t = cpool.tile([128, GW], BF16, tag=f"wv{cc}", name=f"wv{cc}")
                nc.sync.dma_start(t[:], wv[128 * cc:128 * (cc + 1), :])
                wv_sb.append(t)
            for cc in range(CCH):
                for name, dram, lst in (("wq", wq, wq_sb), ("wk", wk, wk_sb)):
                    t = cpool.tile([128, GW], BF16, tag=f"{name}{cc}",
                                   name=f"{name}{cc}")
                    nc.sync.dma_start(t[:], dram[128 * cc:128 * (cc + 1), :])
                    lst.append(t)
            wp_sb = []
            for p in range(NPAIR):
                t = cpool.tile([128, C], BF16, tag=f"wp{p}", name=f"wp{p}")
                nc.sync.dma_start(t[:], wp[128 * p:128 * (p + 1), :])
                wp_sb.append(t)
            negI_sb = cpool.tile([128, 128], BF16, tag="negI", name="negI_t")
            nc.sync.dma_start(negI_sb[:], negI[:, :])
            tri_sb = cpool.tile([128, 896], BF16, tag="tri", name="tri_t")
            nc.sync.dma_start(tri_sb[:], tri[:, :])
            # preload the ACT exp table set while input DMAs run
            actwarm = cpool.tile([1, 8], F32, tag="actwarm", name="actwarm")
            nc.vector.memset(actwarm[:], 0.0)
            nc.scalar.activation(actwarm[:], actwarm[:], EXP)

            # O^T normalized, per head pair: head0 partitions 0:64,
            # head1 partitions 64:128 (layout = rows of w_proj)
            otn_sb = [cpool.tile([128, S], BF16, tag=f"otn{p}", name=f"otn{p}")
                      for p in range(NPAIR)]

            # ---- V = x @ wv in natural [s, d] layout, + ones column ----
            vt_sb = [cpool.tile([128, 65 * HG], BF16, tag=f"vt{sc}",
                                name=f"vt{sc}")
                     for sc in range(SC)]

            def emit_v_strip(sc):
                vt = vt_sb[sc]
                nc.gpsimd.memset(vt[:], 1.0)
                ps = mmpool.tile([128, GW], F32, tag="mm", name="vps")
                for cc in range(CCH):
                    nc.tensor.matmul(
                        ps[:], xt_sb[cc][:, 128 * sc:128 * (sc + 1)],
                        wv_sb[cc][:], start=(cc == 0), stop=(cc == CCH - 1))
                vt_v = vt[:, :].rearrange("p (h d) -> p h d", h=HG)[:, :, 0:64]
                ps_v = ps[:, :].rearrange("p (h d) -> p h d", h=HG)
                nc.vector.tensor_copy(vt_v, ps_v)

            # ---- QK projection mini-chunk: one (pair, q|k, s-tile) ----
            qt_t = [None] * NPAIR
            kt_t = [None] * NPAIR

            def emit_qk_chunk(p, which, st):
                if which == "q":
                    if qt_t[p] is None:
                        qt_t[p] = qkpool.tile([128, S], BF16, tag="qt",
                                              name=f"qt{p}")
                    w_sb, dst = wq_sb, qt_t[p]
                else:
                    if kt_t[p] is None:
                        kt_t[p] = qkpool.tile([128, S], BF16, tag="kt",
                                              name=f"kt{p}")
                    w_sb, dst = wk_sb, kt_t[p]
                ps = mmpool.tile([128, 512], F32, tag="mm", name="qkps")
                for cc in range(CCH):
                    nc.tensor.matmul(
                        ps[:],
                        w_sb[cc][:, 128 * p:128 * (p + 1)],
                        xt_sb[cc][:, 512 * st:512 * (st + 1)],
                        start=(cc == 0), stop=(cc == CCH - 1))
                nc.vector.tensor_copy(dst[:, 512 * st:512 * (st + 1)], ps[:])

            # ---- output projection mini-chunk: one (s-chunk, half) ----
            outst_t = {}

            def emit_outproj(sc, half):
                pp = mmpool.tile([128, 512], F32, tag="mm", name="pp")
                for p in range(NPAIR):
                    nc.tensor.matmul(
                        pp[:],
                        otn_sb[p][:, 128 * sc:128 * (sc + 1)],
                        wp_sb[p][:, 512 * half:512 * (half + 1)],
                        start=(p == 0), stop=(p == NPAIR - 1))
                if sc not in outst_t:
                    outst_t[sc] = wpool.tile([128, C], F32, tag="outst",
                                             name=f"outst{sc}")
                o = outst_t[sc]
                nc.vector.tensor_copy(o[:, 512 * half:512 * (half + 1)], pp[:])
                if half == 1:
                    nc.sync.dma_start(out[128 * sc:128 * (sc + 1), :], o[:])
                    del outst_t[sc]

            # ---- filler queue: (deadline, closure), deadline = (p, j, g)
            fillq = []

            def drain_fillers(now, budget):
                i = 0
                while i < len(fillq):
                    dl, fn = fillq[i]
                    if dl <= now:
                        fn()
                        fillq.pop(i)
                        budget -= 1
                    else:
                        i += 1
                while budget > 0 and fillq:
                    dl, fn = fillq.pop(0)
                    fn()
                    budget -= 1

            # ---- attention inner machinery ----
            def emit_scores_kb(p, j, kb):
                qt, kt = qt_t[p], kt_t[p]
                sp = spspool.tile([128, 1024], F32, tag="sp", name="sp")
                diag = kb >= 4 * j
                ot_ = 128 * (kb - 4 * j) if diag else 0  # true block offset
                o = ot_ if diag_restrict else 0          # written col range
                for h in range(2):
                    hsl = slice(64 * h, 64 * (h + 1))
                    nc.tensor.matmul(
                        sp[:, 512 * h + o:512 * (h + 1)],
                        kt[hsl, 128 * kb:128 * (kb + 1)],
                        qt[hsl, 512 * j + o:512 * (j + 1)],
                        start=True, stop=not diag)
                if diag:
                    # single full-array K=128 matmul: row-tiled halves
                    # would co-execute into the same psum bank, which
                    # the HW forbids. On HW the dead triangle sits entirely
                    # in the 128-col strip [ot_:ot_+128) and has_written is
                    # already set there, so an N=128 accumulate suffices
                    # (stop is sim-only; skip the bass group check). The
                    # sim build closes the full written range instead.
                    if diag_restrict:
                        for h in range(2):
                            csl = slice(512 * h + ot_, 512 * h + ot_ + 128)
                            nc.tensor.matmul(
                                sp[:, csl], negI_sb[:, :],
                                tri_sb[:, 384:512],
                                start=False, stop=True,
                                skip_group_check=True)
                    else:
                        sh = 384 - ot_
                        for h in range(2):
                            csl = slice(512 * h, 512 * (h + 1))
                            nc.tensor.matmul(
                                sp[:, csl], negI_sb[:, :],
                                tri_sb[:, sh:sh + 512],
                                start=False, stop=True)
                pt = ptpool.tile([128, 1024], BF16, tag="pt", name="pt")
                nc.scalar.activation(pt[:], sp[:], EXP, scale=SCALE)
                return pt

            def emit_av_group(p, j, g, pts, ot, nkb):
                for kb in (2 * g, 2 * g + 1):
                    o = 128 * (kb - 4 * j) if kb >= 4 * j else 0
                    for h in range(2):
                        nc.tensor.matmul(
                            ot[h][:, o:512],
                            vt_sb[kb][:, 65 * (2 * p + h):
                                      65 * (2 * p + h) + 65],
                            pts[kb][:, 512 * h + o:512 * (h + 1)],
                            start=(kb == 0), stop=(kb == nkb - 1))

            def emit_normalize(p, j, ot):
                qsl = slice(512 * j, 512 * (j + 1))
                s64 = wpool.tile([65, 1024], F32, tag="s64", name="s64")
                for h in range(2):
                    nc.vector.tensor_copy(s64[:, 512 * h:512 * (h + 1)],
                                          ot[h][:, :])
                # partition-shift the two denominator rows to partition 0
                dsh = wpool.tile([1, 1024], F32, tag="dsh", name="dsh")
                nc.sync.dma_start(dsh[0:1, :], s64[64:65, :])
                inv = wpool.tile([1, 1024], F32, tag="inv", name="inv")
                nc.vector.reciprocal_approx_fast(inv[0:1, :], dsh[0:1, :])
                for h in range(2):
                    bcs = wpool.tile([64, 512], F32, tag="bcs", name="bcs")
                    nc.gpsimd.partition_broadcast(
                        bcs[:], inv[0:1, 512 * h:512 * (h + 1)])
                    if h == 0:
                        nc.vector.tensor_mul(otn_sb[p][0:64, qsl],
                                             s64[0:64, 0:512], bcs[:])
                    else:
                        oth = wpool.tile([64, 512], BF16, tag="oth",
                                         name="oth")
                        nc.vector.tensor_mul(oth[:], s64[0:64, 512:1024],
                                             bcs[:])
                        # partition-shifting copy into rows 64:128
                        nc.sync.dma_start(otn_sb[p][64:128, qsl], oth[:])

            # ---- main schedule ----
            # upfront: V strips 0..3, pair-0 qt/kt for j=0
            for sc in range(4):
                emit_v_strip(sc)
            emit_qk_chunk(0, "q", 0)
            emit_qk_chunk(0, "k", 0)
            # queue the rest of pair 0's QK with deadlines
            for st in range(1, NQT):
                fillq.append(((0, st, 0),
                              lambda st=st: emit_qk_chunk(0, "q", st)))
                fillq.append(((0, st, 0),
                              lambda st=st: emit_qk_chunk(0, "k", st)))

            for p in range(NPAIR):
                # queue next pair's QK chunks (due before (p+1, st, 0))
                if p + 1 < NPAIR:
                    for st in range(NQT):
                        fillq.append(((p + 1, st, 0),
                                      lambda p=p, st=st:
                                      emit_qk_chunk(p + 1, "q", st)))
                        fillq.append(((p + 1, st, 0),
                                      lambda p=p, st=st:
                                      emit_qk_chunk(p + 1, "k", st)))
                if p == 0:
                    # V strips 4..15: strip s first consumed by AV in
                    # (0, j=s//4) at group ~s//2+3 (lag-2 pipeline)
                    for s in range(4, SC):
                        jj = s // 4
                        gg = min(s // 2 + 3, 2 * (jj + 1) - 1)
                        fillq.append(((0, jj, gg),
                                      lambda s=s: emit_v_strip(s)))
                    fillq.sort(key=lambda e: e[0])

                for j in range(NQT):
                    nkb = 4 * (j + 1)
                    ot = [otpool.tile([65, 512], F32, tag="ot", name="ot")
                          for _ in range(2)]
                    pts = {}
                    pending = []
                    for g in range(nkb // 2):
                        drain_fillers((p, j, g), 1)
                        for kb in (2 * g, 2 * g + 1):
                            pts[kb] = emit_scores_kb(p, j, kb)
                        pending.append(g)
                        if len(pending) > 2:
                            emit_av_group(p, j, pending.pop(0), pts, ot, nkb)
                    for g in pending:
                        emit_av_group(p, j, g, pts, ot, nkb)
                    emit_normalize(p, j, ot)
                    if p != 3:
                        drain_fillers((p, j, 99), 1)
                    if p == 3:
                        # out-projection for this j's s-chunks becomes legal
                        # once all pairs have normalized j
                        for sc in range(4 * j, 4 * (j + 1)):
                            for half in range(2):
                                fillq.append(
                                    (END, lambda sc=sc, half=half:
                                     emit_outproj(sc, half)))
            drain_fillers(END, len(fillq) + 1)

    nc.compile()
    return nc


_NC_CACHE = None


def _get_nc():
    global _NC_CACHE
    if _NC_CACHE is None:
        _NC_CACHE = build_nc()
    return _NC_CACHE


def make_in_maps(x, w_qkv, w_proj):
    """Shard full inputs into the 8 per-core input dicts."""
    bf = ml_dtypes.bfloat16
    negI = (NEG * np.eye(128, dtype=np.float32)).astype(bf)
    tri = ((np.arange(896)[None, :] - 384) < np.arange(128)[:, None]).astype(bf)
    in_maps = []
    for core in range(N_CORES):
        b, g = core // 2, core % 2
        gsl = slice(GW * g, GW * (g + 1))
        in_maps.append({
            "xT": np.ascontiguousarray(x[b].T).astype(bf),
            "wq": np.ascontiguousarray(w_qkv[:, 0 * C:1 * C][:, gsl]).astype(bf),
            "wk": np.ascontiguousarray(w_qkv[:, 1 * C:2 * C][:, gsl]).astype(bf),
            "wv": np.ascontiguousarray(w_qkv[:, 2 * C:3 * C][:, gsl]).astype(bf),
            "wp": np.ascontiguousarray(w_proj[gsl, :]).astype(bf),
            "negI": negI,
            "tri": tri,
        })
    return in_maps


def kernel(x, w_qkv, w_proj, b_proj, _profile=False):
    import os
    if not _profile:
        # the NTFF trace path needs modules absent from this image;
        # make sure an inherited BASS_TRACE can't route us into it
        os.environ["BASS_NEVER_TRACE"] = "1"
    else:
        os.environ.pop("BASS_NEVER_TRACE", None)
    x = np.asarray(x, np.float32)
    w_qkv = np.asarray(w_qkv, np.float32)
    w_proj = np.asarray(w_proj, np.float32)
    b_proj = np.asarray(b_proj, np.float32)

    nc = _get_nc()
    in_maps = make_in_maps(x, w_qkv, w_proj)
    res = run_bass_kernel_spmd(nc, in_maps, core_ids=list(range(N_CORES)),
                               trace=_profile)
    partials = [res.results[c]["out"] for c in range(N_CORES)]
    out = np.empty((B, S, C), np.float32)
    for b in range(B):
        out[b] = partials[2 * b] + partials[2 * b + 1] + b_proj
    if _profile:
        return out, res
    return out
